# revision 27
# baseline (speedup 1.0000x reference)
"""Trainium2 Bass kernel for PointerAttention (additive/Bahdanau attention scores).

Computes, for full inputs:
    w1d = dec @ W1 + b1                    # [B, Td, U]
    w2e = enc @ W2 + b2                    # [B, Te, U]
    score[b,t,s] = V . tanh(w1d[b,t] + w2e[b,s]) (+ bv, dropped: softmax-shift-invariant)
    out = softmax(score, axis=-1)          # [B, Td, Te]

Shapes: B=16, Td=128, Te=1024, H=256, U=128.

Strategy (8 NeuronCores, data-parallel over B, 2 batches/core):
  - Host pre-transposes dec/enc to [B, H, *] so the contraction dim lands on
    SBUF partitions; weights stay natural ([H, U] == lhsT layout).
  - On-device projections produce w2eT [u, s] and w1dT [u, t] (+b1+b2 folded).
  - Main loop per t: DVE adds w1dT[:, t] (per-partition scalar, 4x mode, bf16)
    onto w2eT; ScalarE tanh in-place over T_BLK t's per instruction; TensorE
    contracts with V via 32-wide zero-padded V-column stationaries so each t's
    score row accumulates into a distinct PSUM partition (row r = 32*(t%4)+t//4).
  - Softmax per 128-row block: DVE -max, ScalarE exp(+bias) with accumulated
    row sums, DVE reciprocal + per-partition scale; output DMA un-permutes rows.
"""

import numpy as np

B, Td, Te, H, U = 16, 128, 1024, 256, 128
NCORES = 8
BPC = B // NCORES  # batches per core
KC = H // 128      # contraction chunks

# tanh batching: ACT instruction overhead is ~352 cycles, so batch many t's
# per instruction. b=0 ramps up fine-grained so ScalarE starts ASAP; the last
# batch tapers down so the final score matmuls don't delay the exp.
_BLOCKS_B0 = [2, 2, 4, 8, 16, 16, 16, 16, 16, 16, 16]
_BLOCKS_B1 = [16, 16, 16, 16, 16, 16, 16, 8, 4, 2, 2]
_SPLIT_HALF_T = 4  # adds+tanh split per s-half for t < this (b=0 only)

_NC_CACHE = {}


def _build_nc():
    if "nc" in _NC_CACHE:
        return _NC_CACHE["nc"]

    from contextlib import ExitStack

    import concourse.bacc as bacc
    import concourse.tile as tile
    from concourse import mybir

    f32 = mybir.dt.float32
    bf16 = mybir.dt.bfloat16
    AF = mybir.ActivationFunctionType

    nc = bacc.Bacc()
    # host pre-lays contraction dim as [partition, chunk]: h = c*128 + p
    encT = nc.dram_tensor("enct", [BPC, 128, KC, Te], f32, kind="ExternalInput")
    decT = nc.dram_tensor("dect", [BPC, 128, KC, Td], f32, kind="ExternalInput")
    w1 = nc.dram_tensor("w1", [128, KC, U], f32, kind="ExternalInput")
    w2 = nc.dram_tensor("w2", [128, KC, U], f32, kind="ExternalInput")
    b12 = nc.dram_tensor("b12", [U, 1], f32, kind="ExternalInput")
    vcols = nc.dram_tensor("vcols", [U, 32, 32], bf16, kind="ExternalInput")
    out = nc.dram_tensor("out", [BPC, Td, Te], f32, kind="ExternalOutput")

    with tile.TileContext(nc) as tc, ExitStack() as ctx:
        singles = ctx.enter_context(tc.tile_pool(name="singles", bufs=1))
        proj_in = ctx.enter_context(tc.tile_pool(name="proj_in", bufs=2))
        proj_ps = ctx.enter_context(tc.tile_pool(name="proj_ps", bufs=2, space="PSUM"))
        w2e_pool = ctx.enter_context(tc.tile_pool(name="w2e", bufs=2))
        w1d_pool = ctx.enter_context(tc.tile_pool(name="w1d", bufs=2))
        feat_pool = ctx.enter_context(tc.tile_pool(name="feat", bufs=3))
        score_pool = ctx.enter_context(tc.tile_pool(name="score", bufs=2, space="PSUM"))
        sm_pool = ctx.enter_context(tc.tile_pool(name="sm", bufs=4))
        prob_pool = ctx.enter_context(tc.tile_pool(name="prob", bufs=2))
        poly_pool = ctx.enter_context(tc.tile_pool(name="poly", bufs=2))

        w1_sb = singles.tile([128, KC, U], f32)
        w2_sb = singles.tile([128, KC, U], f32)
        b12_sb = singles.tile([128, 1], f32)
        vcols_sb = singles.tile([128, 32, 32], bf16)

        # --- input DMAs, one per tensor piece. Descriptor-gen (~0.65us) is
        # serial per issuing engine, so spread the head-critical ones across
        # idle engine queues: enc b0 gates the first tanh. ---
        dec_sbs, enc_sbs = [], []
        for b in range(BPC):
            dec_sb = proj_in.tile([128, KC, Td], f32, tag="dec")
            dec_sbs.append(dec_sb)
            enc_sb = proj_in.tile([128, KC, Te], f32, tag="enc")
            enc_sbs.append(enc_sb)

        def enc_dma(eng, b, h):
            eng.dma_start(
                out=enc_sbs[b][:, :, h * 512:(h + 1) * 512],
                in_=encT[b, :, :, h * 512:(h + 1) * 512],
            )

        enc_dma(nc.gpsimd, 0, 0)
        enc_dma(nc.scalar, 0, 1)
        nc.sync.dma_start(out=w1_sb, in_=w1[:, :, :])
        nc.sync.dma_start(out=dec_sbs[0], in_=decT[0])
        nc.sync.dma_start(out=w2_sb, in_=w2[:, :, :])
        nc.sync.dma_start(out=dec_sbs[1], in_=decT[1])
        nc.sync.dma_start(out=b12_sb, in_=b12[:, :])
        nc.sync.dma_start(out=vcols_sb, in_=vcols[:, :, :])
        enc_dma(nc.sync, 1, 0)
        enc_dma(nc.sync, 1, 1)

        # --- projections: w2eT [u, s] bf16, w1dT [u, t] f32 (+b1+b2 folded
        # into the PSUM->SBUF copy so ScalarE only ever runs tanh/exp) ---
        ADD, MUL = mybir.AluOpType.add, mybir.AluOpType.mult
        MAXO, MINO = mybir.AluOpType.max, mybir.AluOpType.min
        w1dTs, w2eTs = [None] * BPC, [None] * BPC

        def emit_proj(b, keep_ps):
            w1dT = w1d_pool.tile([128, Td], f32, tag="w1dT")
            dps = proj_ps.tile([128, Td], f32, tag="decps")
            for c in range(KC):
                nc.tensor.matmul(
                    dps, w1_sb[:, c, :], dec_sbs[b][:, c, :],
                    start=(c == 0), stop=(c == KC - 1),
                )
            nc.vector.tensor_scalar_add(w1dT, dps, b12_sb[:, 0:1])
            w1dTs[b] = w1dT
            w2eT = w2e_pool.tile([128, Te], bf16, tag="w2eT")
            w2eTs[b] = w2eT
            ps_list = []
            for h in range(Te // 512):
                ps = proj_ps.tile([128, 512], f32, tag="encps")
                for c in range(KC):
                    nc.tensor.matmul(
                        ps, w2_sb[:, c, :], enc_sbs[b][:, c, h * 512:(h + 1) * 512],
                        start=(c == 0), stop=(c == KC - 1),
                    )
                nc.vector.tensor_copy(w2eT[:, h * 512:(h + 1) * 512], ps)
                ps_list.append(ps)
            return ps_list if keep_ps else None

        def poly_tanh(feat_sl, w2eT, w1d_col):
            # rational tanh on DVE (bf16): y = x(n1+n3 t+n5 t^2)/(d0+d2 t+d4 t^2+d6 t^3)
            C = 4.0
            N1, N3, N5 = 1.68930297, 0.199634309, 0.00296604931
            D0, D2, D4, D6 = 1.68930496, 0.762726864, 0.0319797329, 0.000123647857
            xh = poly_pool.tile([128, Te], bf16, tag="xh")
            t2 = poly_pool.tile([128, Te], bf16, tag="t2")
            den = poly_pool.tile([128, Te], bf16, tag="den")
            V = nc.vector
            V.tensor_scalar(xh, w2eT, w1d_col, C, op0=ADD, op1=MINO)
            V.tensor_scalar(xh, xh, -C, None, op0=MAXO)
            V.tensor_mul(t2, xh, xh)
            V.tensor_scalar(den, t2, D6, D4, op0=MUL, op1=ADD)
            V.tensor_mul(den, den, t2)
            V.tensor_scalar(den, den, D2, None, op0=ADD)
            V.tensor_mul(den, den, t2)
            V.tensor_scalar(den, den, D0, None, op0=ADD)
            with nc.allow_low_precision(reason="bf16 rational tanh"):
                V.reciprocal(den, den)
            V.tensor_scalar(feat_sl, t2, N5, N3, op0=MUL, op1=ADD)
            V.tensor_mul(feat_sl, feat_sl, t2)
            V.tensor_scalar(feat_sl, feat_sl, N1, None, op0=ADD)
            V.tensor_mul(feat_sl, feat_sl, xh)
            V.tensor_mul(feat_sl, feat_sl, den)

        def emit_score_mms(sc_ps, feat, t0, blk_sz):
            for tt in range(blk_sz):
                t = t0 + tt
                g, r = t % 4, t // 4
                for hh in range(Te // 512):
                    nc.tensor.matmul(
                        sc_ps[32 * g:32 * (g + 1), hh * 512:(hh + 1) * 512],
                        vcols_sb[:, r, :],
                        feat[:, tt, hh * 512:(hh + 1) * 512],
                        start=(t < 4),
                        stop=(t >= Td - 4),
                        tile_position=(0, 32 * g),
                        skip_group_check=True,
                    )

        def emit_block(b, sc_ps, t0, blk_sz, head_ps=None):
            w1dT, w2eT = w1dTs[b], w2eTs[b]
            feat = feat_pool.tile([128, blk_sz, Te], bf16, tag="feat")
            if head_ps is not None:
                # head: tanh straight off the projection PSUM with the
                # per-partition bias add fused into the activation
                for hh in range(2):
                    for tt in range(blk_sz):
                        nc.scalar.activation(
                            feat[:, tt, hh * 512:(hh + 1) * 512],
                            head_ps[hh],
                            AF.Tanh,
                            bias=w1dT[:, t0 + tt:t0 + tt + 1],
                        )
            elif blk_sz == 16:
                # steady block: 15 t's on ScalarE, last t on DVE (rational)
                for tt in range(15):
                    nc.vector.tensor_scalar_add(
                        feat[:, tt, :], w2eT, w1dT[:, t0 + tt:t0 + tt + 1]
                    )
                nc.scalar.activation(feat[:, 0:15, :], feat[:, 0:15, :], AF.Tanh)
                poly_tanh(feat[:, 15, :], w2eT, w1dT[:, t0 + 15:t0 + 16])
            else:
                for tt in range(blk_sz):
                    nc.vector.tensor_scalar_add(
                        feat[:, tt, :], w2eT, w1dT[:, t0 + tt:t0 + tt + 1]
                    )
                nc.scalar.activation(feat, feat, AF.Tanh)
            emit_score_mms(sc_ps, feat, t0, blk_sz)

        def emit_softmax_out(b, sc_ps):
            # scores are bounded (|score| <= sum|V| ~ 10): exp needs no max sub
            probs = prob_pool.tile([128, Te], f32, tag="probs")
            sums = sm_pool.tile([128, 1], f32, tag="sums")
            nc.scalar.activation(probs, sc_ps, AF.Exp, accum_out=sums[:, 0:1])
            rinv = sm_pool.tile([128, 1], f32, tag="rinv")
            nc.vector.reciprocal(rinv, sums)
            nc.vector.tensor_scalar_mul(probs, probs, rinv[:, 0:1])
            # un-permuting output DMA: partition r=32g+c holds t=4c+g
            out_gcs = out[b].rearrange("(c g) s -> g c s", g=4)
            for g, eng in enumerate((nc.sync, nc.gpsimd, nc.sync, nc.gpsimd)):
                eng.dma_start(out=out_gcs[g], in_=probs[32 * g:32 * (g + 1), :])

        # b0 proj + head blocks first (head reads proj PSUM directly), then
        # b1 proj (hidden under b0 main loop), then the steady blocks.
        ps_b0 = emit_proj(0, keep_ps=True)
        sc_ps0 = score_pool.tile([128, Te], f32, tag="scps")
        t = 0
        for blk_sz in _BLOCKS_B0:
            if t < _SPLIT_HALF_T:
                emit_block(0, sc_ps0, t, blk_sz, head_ps=ps_b0)
                t += blk_sz
                if t >= _SPLIT_HALF_T:
                    emit_proj(1, keep_ps=False)
            else:
                emit_block(0, sc_ps0, t, blk_sz)
                t += blk_sz
        emit_softmax_out(0, sc_ps0)

        sc_ps1 = score_pool.tile([128, Te], f32, tag="scps")
        t = 0
        for blk_sz in _BLOCKS_B1:
            emit_block(1, sc_ps1, t, blk_sz)
            t += blk_sz
        emit_softmax_out(1, sc_ps1)

    nc.finalize()
    _NC_CACHE["nc"] = nc
    return nc


def _prep_shards(dec_outputs, enc_outputs, W1, b1, W2, b2, V, bv):
    import ml_dtypes

    # [B, T, H] -> [B, 128(p), KC(c), T] with h = c*128 + p
    dec = np.ascontiguousarray(
        np.asarray(dec_outputs, np.float32)
        .transpose(0, 2, 1).reshape(B, KC, 128, Td).transpose(0, 2, 1, 3)
    )
    enc = np.ascontiguousarray(
        np.asarray(enc_outputs, np.float32)
        .transpose(0, 2, 1).reshape(B, KC, 128, Te).transpose(0, 2, 1, 3)
    )
    # [H, U] -> [128(p), KC(c), U]
    w1 = np.ascontiguousarray(np.asarray(W1, np.float32).reshape(KC, 128, U).transpose(1, 0, 2))
    w2 = np.ascontiguousarray(np.asarray(W2, np.float32).reshape(KC, 128, U).transpose(1, 0, 2))
    b12 = (np.asarray(b1, np.float32) + np.asarray(b2, np.float32)).reshape(U, 1)
    v = np.asarray(V, np.float32).reshape(U)
    vcols = np.zeros((U, 32, 32), np.float32)
    for r in range(32):
        vcols[:, r, r] = v
    vcols = vcols.astype(ml_dtypes.bfloat16)
    # bv dropped: softmax(score + const) == softmax(score).
    in_maps = []
    for i in range(NCORES):
        in_maps.append({
            "enct": np.ascontiguousarray(enc[i * BPC:(i + 1) * BPC]),
            "dect": np.ascontiguousarray(dec[i * BPC:(i + 1) * BPC]),
            "w1": w1,
            "w2": w2,
            "b12": b12,
            "vcols": vcols,
        })
    return in_maps


def kernel(dec_outputs, enc_outputs, W1, b1, W2, b2, V, bv):
    from concourse.bass_utils import run_bass_kernel_spmd

    nc = _build_nc()
    in_maps = _prep_shards(dec_outputs, enc_outputs, W1, b1, W2, b2, V, bv)
    res = run_bass_kernel_spmd(nc, in_maps, list(range(NCORES))).results
    out = np.concatenate([np.asarray(res[i]["out"]) for i in range(NCORES)], axis=0)
    return np.ascontiguousarray(out.astype(np.float32))


# revision 29
# speedup vs baseline: 1.2532x; 1.2532x over previous
"""Trainium2 Bass kernel for PointerAttention (additive/Bahdanau attention scores).

Computes, for full inputs:
    w1d = dec @ W1 + b1                    # [B, Td, U]
    w2e = enc @ W2 + b2                    # [B, Te, U]
    score[b,t,s] = V . tanh(w1d[b,t] + w2e[b,s]) (+ bv, dropped: softmax-shift-invariant)
    out = softmax(score, axis=-1)          # [B, Td, Te]

Shapes: B=16, Td=128, Te=1024, H=256, U=128.

Strategy (8 NeuronCores, data-parallel over B, 2 batches/core):
  - Host pre-transposes dec/enc to [B, H, *] so the contraction dim lands on
    SBUF partitions; weights stay natural ([H, U] == lhsT layout).
  - On-device projections produce w2eT [u, s] and w1dT [u, t] (+b1+b2 folded).
  - Main loop per t: DVE adds w1dT[:, t] (per-partition scalar, 4x mode, bf16)
    onto w2eT; ScalarE tanh in-place over T_BLK t's per instruction; TensorE
    contracts with V via 32-wide zero-padded V-column stationaries so each t's
    score row accumulates into a distinct PSUM partition (row r = 32*(t%4)+t//4).
  - Softmax per 128-row block: DVE -max, ScalarE exp(+bias) with accumulated
    row sums, DVE reciprocal + per-partition scale; output DMA un-permutes rows.
"""

import numpy as np

B, Td, Te, H, U = 16, 128, 1024, 256, 128
NCORES = 8
BPC = B // NCORES  # batches per core
KC = H // 128      # contraction chunks

# tanh batching: ACT instruction overhead is ~352 cycles, so batch many t's
# per instruction. b=0 ramps up fine-grained so ScalarE starts ASAP; the last
# batch tapers down so the final score matmuls don't delay the exp.
_BLOCKS_B0 = [2, 2, 4, 8, 16, 16, 16, 16, 16, 16, 16]
_BLOCKS_B1 = [16, 16, 16, 16, 16, 16, 16, 8, 4, 2, 2]
_SPLIT_HALF_T = 4  # adds+tanh split per s-half for t < this (b=0 only)

_NC_CACHE = {}


def _build_nc():
    if "nc" in _NC_CACHE:
        return _NC_CACHE["nc"]

    from contextlib import ExitStack

    import concourse.bacc as bacc
    import concourse.tile as tile
    from concourse import mybir

    f32 = mybir.dt.float32
    bf16 = mybir.dt.bfloat16
    AF = mybir.ActivationFunctionType

    nc = bacc.Bacc()
    # host pre-lays contraction dim as [partition, chunk]: h = c*128 + p
    encT = nc.dram_tensor("enct", [BPC, 128, KC, Te], f32, kind="ExternalInput")
    decT = nc.dram_tensor("dect", [BPC, 128, KC, Td], f32, kind="ExternalInput")
    w1 = nc.dram_tensor("w1", [128, KC, U], f32, kind="ExternalInput")
    w2 = nc.dram_tensor("w2", [128, KC, U], f32, kind="ExternalInput")
    b12 = nc.dram_tensor("b12", [U, 1], f32, kind="ExternalInput")
    vcols = nc.dram_tensor("vcols", [U, 32, 32], bf16, kind="ExternalInput")
    out = nc.dram_tensor("out", [BPC, Td, Te], f32, kind="ExternalOutput")

    with tile.TileContext(nc) as tc, ExitStack() as ctx:
        singles = ctx.enter_context(tc.tile_pool(name="singles", bufs=1))
        proj_in = ctx.enter_context(tc.tile_pool(name="proj_in", bufs=2))
        proj_ps = ctx.enter_context(tc.tile_pool(name="proj_ps", bufs=2, space="PSUM"))
        w2e_pool = ctx.enter_context(tc.tile_pool(name="w2e", bufs=2))
        w1d_pool = ctx.enter_context(tc.tile_pool(name="w1d", bufs=2))
        feat_pool = ctx.enter_context(tc.tile_pool(name="feat", bufs=3))
        score_pool = ctx.enter_context(tc.tile_pool(name="score", bufs=2, space="PSUM"))
        sm_pool = ctx.enter_context(tc.tile_pool(name="sm", bufs=4))
        prob_pool = ctx.enter_context(tc.tile_pool(name="prob", bufs=2))
        poly_pool = ctx.enter_context(tc.tile_pool(name="poly", bufs=2))

        w1_sb = singles.tile([128, KC, U], f32)
        w2_sb = singles.tile([128, KC, U], f32)
        b12_sb = singles.tile([128, 1], f32)
        vcols_sb = singles.tile([128, 32, 32], bf16)

        # --- input DMAs, one per tensor piece. Descriptor-gen (~0.65us) is
        # serial per issuing engine, so spread the head-critical ones across
        # idle engine queues: enc b0 gates the first tanh. ---
        dec_sbs, enc_sbs = [], []
        for b in range(BPC):
            dec_sb = proj_in.tile([128, KC, Td], f32, tag="dec")
            dec_sbs.append(dec_sb)
            enc_sb = proj_in.tile([128, KC, Te], f32, tag="enc")
            enc_sbs.append(enc_sb)

        def enc_dma(eng, b, h):
            eng.dma_start(
                out=enc_sbs[b][:, :, h * 512:(h + 1) * 512],
                in_=encT[b, :, :, h * 512:(h + 1) * 512],
            )

        enc_dma(nc.gpsimd, 0, 0)
        enc_dma(nc.scalar, 0, 1)
        nc.sync.dma_start(out=w1_sb, in_=w1[:, :, :])
        nc.sync.dma_start(out=dec_sbs[0], in_=decT[0])
        nc.sync.dma_start(out=w2_sb, in_=w2[:, :, :])
        nc.sync.dma_start(out=dec_sbs[1], in_=decT[1])
        nc.sync.dma_start(out=b12_sb, in_=b12[:, :])
        nc.sync.dma_start(out=vcols_sb, in_=vcols[:, :, :])
        enc_dma(nc.sync, 1, 0)
        enc_dma(nc.sync, 1, 1)

        # --- projections: w2eT [u, s] bf16, w1dT [u, t] f32 (+b1+b2 folded
        # into the PSUM->SBUF copy so ScalarE only ever runs tanh/exp) ---
        ADD, MUL = mybir.AluOpType.add, mybir.AluOpType.mult
        MAXO, MINO = mybir.AluOpType.max, mybir.AluOpType.min
        w1dTs, w2eTs = [None] * BPC, [None] * BPC

        def emit_proj(b, keep_ps):
            w1dT = w1d_pool.tile([128, Td], f32, tag="w1dT")
            dps = proj_ps.tile([128, Td], f32, tag="decps")
            for c in range(KC):
                nc.tensor.matmul(
                    dps, w1_sb[:, c, :], dec_sbs[b][:, c, :],
                    start=(c == 0), stop=(c == KC - 1),
                )
            nc.vector.tensor_scalar_add(w1dT, dps, b12_sb[:, 0:1])
            w1dTs[b] = w1dT
            w2eT = w2e_pool.tile([128, Te], bf16, tag="w2eT")
            w2eTs[b] = w2eT
            ps_list = []
            for h in range(Te // 512):
                ps = proj_ps.tile([128, 512], f32, tag="encps")
                for c in range(KC):
                    nc.tensor.matmul(
                        ps, w2_sb[:, c, :], enc_sbs[b][:, c, h * 512:(h + 1) * 512],
                        start=(c == 0), stop=(c == KC - 1),
                    )
                nc.vector.tensor_copy(w2eT[:, h * 512:(h + 1) * 512], ps)
                ps_list.append(ps)
            return ps_list if keep_ps else None

        def poly_tanh(feat_sl, w2eT, w1d_col):
            # rational tanh on DVE (bf16): y = x(n1+n3 t+n5 t^2)/(d0+d2 t+d4 t^2+d6 t^3)
            C = 4.0
            N1, N3, N5 = 1.68930297, 0.199634309, 0.00296604931
            D0, D2, D4, D6 = 1.68930496, 0.762726864, 0.0319797329, 0.000123647857
            xh = poly_pool.tile([128, Te], bf16, tag="xh")
            t2 = poly_pool.tile([128, Te], bf16, tag="t2")
            den = poly_pool.tile([128, Te], bf16, tag="den")
            V = nc.vector
            V.tensor_scalar(xh, w2eT, w1d_col, C, op0=ADD, op1=MINO)
            V.tensor_scalar(xh, xh, -C, None, op0=MAXO)
            V.tensor_mul(t2, xh, xh)
            V.tensor_scalar(den, t2, D6, D4, op0=MUL, op1=ADD)
            V.tensor_mul(den, den, t2)
            V.tensor_scalar(den, den, D2, None, op0=ADD)
            V.tensor_mul(den, den, t2)
            V.tensor_scalar(den, den, D0, None, op0=ADD)
            with nc.allow_low_precision(reason="bf16 rational tanh"):
                V.reciprocal(den, den)
            V.tensor_scalar(feat_sl, t2, N5, N3, op0=MUL, op1=ADD)
            V.tensor_mul(feat_sl, feat_sl, t2)
            V.tensor_scalar(feat_sl, feat_sl, N1, None, op0=ADD)
            V.tensor_mul(feat_sl, feat_sl, xh)
            V.tensor_mul(feat_sl, feat_sl, den)

        def emit_score_mms(sc_ps, feat, t0, blk_sz):
            for tt in range(blk_sz):
                t = t0 + tt
                g, r = t % 4, t // 4
                for hh in range(Te // 512):
                    nc.tensor.matmul(
                        sc_ps[32 * g:32 * (g + 1), hh * 512:(hh + 1) * 512],
                        vcols_sb[:, r, :],
                        feat[:, tt, hh * 512:(hh + 1) * 512],
                        start=(t < 4),
                        stop=(t >= Td - 4),
                        tile_position=(0, 32 * g),
                        skip_group_check=True,
                    )

        def emit_block(b, sc_ps, t0, blk_sz, head_ps=None):
            w1dT, w2eT = w1dTs[b], w2eTs[b]
            feat = feat_pool.tile([128, blk_sz, Te], bf16, tag="feat")
            if head_ps is not None:
                # head: tanh straight off the projection PSUM with the
                # per-partition bias add fused into the activation
                for hh in range(2):
                    for tt in range(blk_sz):
                        nc.scalar.activation(
                            feat[:, tt, hh * 512:(hh + 1) * 512],
                            head_ps[hh],
                            AF.Tanh,
                            bias=w1dT[:, t0 + tt:t0 + tt + 1],
                        )
            else:
                for tt in range(blk_sz):
                    nc.vector.tensor_scalar_add(
                        feat[:, tt, :], w2eT, w1dT[:, t0 + tt:t0 + tt + 1]
                    )
                nc.scalar.activation(feat, feat, AF.Tanh)
            emit_score_mms(sc_ps, feat, t0, blk_sz)

        def emit_softmax_out(b, sc_ps):
            # scores are bounded (|score| <= sum|V| ~ 10): exp needs no max sub
            probs = prob_pool.tile([128, Te], f32, tag="probs")
            sums = sm_pool.tile([128, 1], f32, tag="sums")
            nc.scalar.activation(probs, sc_ps, AF.Exp, accum_out=sums[:, 0:1])
            rinv = sm_pool.tile([128, 1], f32, tag="rinv")
            nc.vector.reciprocal(rinv, sums)
            nc.vector.tensor_scalar_mul(probs, probs, rinv[:, 0:1])
            # un-permuting output DMA: partition r=32g+c holds t=4c+g
            out_gcs = out[b].rearrange("(c g) s -> g c s", g=4)
            for g, eng in enumerate((nc.sync, nc.gpsimd, nc.sync, nc.gpsimd)):
                eng.dma_start(out=out_gcs[g], in_=probs[32 * g:32 * (g + 1), :])

        # b0 proj + head blocks first (head reads proj PSUM directly), then
        # b1 proj (hidden under b0 main loop), then the steady blocks.
        ps_b0 = emit_proj(0, keep_ps=True)
        sc_ps0 = score_pool.tile([128, Te], f32, tag="scps")
        t = 0
        for blk_sz in _BLOCKS_B0:
            emit_block(0, sc_ps0, t, blk_sz, head_ps=ps_b0 if t < 2 else None)
            t += blk_sz
            if t == 4:
                emit_proj(1, keep_ps=False)
        emit_softmax_out(0, sc_ps0)

        sc_ps1 = score_pool.tile([128, Te], f32, tag="scps")
        t = 0
        for blk_sz in _BLOCKS_B1:
            emit_block(1, sc_ps1, t, blk_sz)
            t += blk_sz
        emit_softmax_out(1, sc_ps1)

    nc.finalize()
    _NC_CACHE["nc"] = nc
    return nc


def _prep_shards(dec_outputs, enc_outputs, W1, b1, W2, b2, V, bv):
    import ml_dtypes

    # [B, T, H] -> [B, 128(p), KC(c), T] with h = c*128 + p
    dec = np.ascontiguousarray(
        np.asarray(dec_outputs, np.float32)
        .transpose(0, 2, 1).reshape(B, KC, 128, Td).transpose(0, 2, 1, 3)
    )
    enc = np.ascontiguousarray(
        np.asarray(enc_outputs, np.float32)
        .transpose(0, 2, 1).reshape(B, KC, 128, Te).transpose(0, 2, 1, 3)
    )
    # [H, U] -> [128(p), KC(c), U]
    w1 = np.ascontiguousarray(np.asarray(W1, np.float32).reshape(KC, 128, U).transpose(1, 0, 2))
    w2 = np.ascontiguousarray(np.asarray(W2, np.float32).reshape(KC, 128, U).transpose(1, 0, 2))
    b12 = (np.asarray(b1, np.float32) + np.asarray(b2, np.float32)).reshape(U, 1)
    v = np.asarray(V, np.float32).reshape(U)
    vcols = np.zeros((U, 32, 32), np.float32)
    for r in range(32):
        vcols[:, r, r] = v
    vcols = vcols.astype(ml_dtypes.bfloat16)
    # bv dropped: softmax(score + const) == softmax(score).
    in_maps = []
    for i in range(NCORES):
        in_maps.append({
            "enct": np.ascontiguousarray(enc[i * BPC:(i + 1) * BPC]),
            "dect": np.ascontiguousarray(dec[i * BPC:(i + 1) * BPC]),
            "w1": w1,
            "w2": w2,
            "b12": b12,
            "vcols": vcols,
        })
    return in_maps


def kernel(dec_outputs, enc_outputs, W1, b1, W2, b2, V, bv):
    from concourse.bass_utils import run_bass_kernel_spmd

    nc = _build_nc()
    in_maps = _prep_shards(dec_outputs, enc_outputs, W1, b1, W2, b2, V, bv)
    res = run_bass_kernel_spmd(nc, in_maps, list(range(NCORES))).results
    out = np.concatenate([np.asarray(res[i]["out"]) for i in range(NCORES)], axis=0)
    return np.ascontiguousarray(out.astype(np.float32))


# revision 34
# speedup vs baseline: 1.4824x; 1.1829x over previous
"""Trainium2 Bass kernel for PointerAttention (additive/Bahdanau attention scores).

Computes, for full inputs:
    w1d = dec @ W1 + b1                    # [B, Td, U]
    w2e = enc @ W2 + b2                    # [B, Te, U]
    score[b,t,s] = V . tanh(w1d[b,t] + w2e[b,s]) (+ bv, dropped: softmax-shift-invariant)
    out = softmax(score, axis=-1)          # [B, Td, Te]

Shapes: B=16, Td=128, Te=1024, H=256, U=128. Data-parallel over B on 8 cores.

Architecture: Fourier separation. tanh(z) on |z|<=8.8 is approximated to
~1e-3 by a sine series sum_k c_k sin(k*w*z) (w = pi/11, K=14). The angle
addition formula sin(kw(d+e)) = sin(kwd)cos(kwe) + cos(kwd)sin(kwe)
separates decoder and encoder dependencies, so

    score[t,s] = sum_k [ (c_k V sin_k(d))^T cos_k(e) + (c_k V cos_k(d))^T sin_k(e) ]

is a sum of 2K true [128u] contractions on the TensorEngine with full array
utilization — no per-(t,s) elementwise work at all. ScalarE computes only the
base sin(w*x), sin(w*x/2) of the projections (straight off the projection
PSUM, scale/bias fused); higher harmonics come from the Chebyshev recurrence
x_{k+1} = 2cos1*x_k - x_{k-1} on the VectorEngine in fp32 (bf16 recurrence
would amplify rounding ~1.9x/step), cast to bf16 only at the matmul inputs.
Softmax: scores are bounded (|score| <= sum|V| ~ 10) so exp needs no max
subtraction; row sums ride along via the activation accumulator.
"""

import numpy as np

B, Td, Te, H, U = 16, 128, 1024, 256, 128
NCORES = 8
BPC = B // NCORES  # batches per core
KC = H // 128      # contraction chunks

FL = 11.0          # sine series half-period
FK = 14            # number of harmonics
FW = float(np.pi / FL)
# minimax-ish LSQ fit of tanh on [0, 8.8] in basis sin(k*FW*z) (see notes)
_COEF = None


def _fit_coeffs():
    global _COEF
    if _COEF is None:
        z = np.linspace(0, 8.8, 12000)
        A = np.sin(np.outer(z, FW * np.arange(1, FK + 1)))
        wt = np.ones_like(z)
        for _ in range(60):
            c, *_ = np.linalg.lstsq(A * wt[:, None], np.tanh(z) * wt, rcond=None)
            err = A @ c - np.tanh(z)
            wt = wt * (1 + 1.5 * (np.abs(err) / np.abs(err).max()) ** 2)
            wt /= wt.mean()
        _COEF = [float(x) for x in c]
    return _COEF


_NC_CACHE = {}


def _build_nc():
    if "nc" in _NC_CACHE:
        return _NC_CACHE["nc"]

    from contextlib import ExitStack

    import concourse.bacc as bacc
    import concourse.tile as tile
    from concourse import mybir

    f32 = mybir.dt.float32
    bf16 = mybir.dt.bfloat16
    AF = mybir.ActivationFunctionType
    ADD, MUL = mybir.AluOpType.add, mybir.AluOpType.mult
    SUB = mybir.AluOpType.subtract
    coef = _fit_coeffs()

    nc = bacc.Bacc()
    # host pre-lays contraction dim as [partition, chunk]: h = c*128 + p
    encT = nc.dram_tensor("enct", [BPC, 128, KC, Te], f32, kind="ExternalInput")
    decT = nc.dram_tensor("dect", [BPC, 128, KC, Td], f32, kind="ExternalInput")
    w1 = nc.dram_tensor("w1", [128, KC, U], f32, kind="ExternalInput")
    w2 = nc.dram_tensor("w2", [128, KC, U], f32, kind="ExternalInput")
    b12 = nc.dram_tensor("b12", [U, 1], f32, kind="ExternalInput")
    vcol = nc.dram_tensor("vcol", [U, 1], f32, kind="ExternalInput")
    out = nc.dram_tensor("out", [BPC, Td, Te], f32, kind="ExternalOutput")

    with tile.TileContext(nc) as tc, ExitStack() as ctx:
        singles = ctx.enter_context(tc.tile_pool(name="singles", bufs=1))
        proj_in = ctx.enter_context(tc.tile_pool(name="proj_in", bufs=2))
        proj_ps = ctx.enter_context(tc.tile_pool(name="proj_ps", bufs=2, space="PSUM"))
        base_pool = ctx.enter_context(tc.tile_pool(name="base", bufs=2))
        echain = ctx.enter_context(tc.tile_pool(name="echain", bufs=3))
        dchain = ctx.enter_context(tc.tile_pool(name="dchain", bufs=3))
        ebf = ctx.enter_context(tc.tile_pool(name="ebf", bufs=4))
        dbf = ctx.enter_context(tc.tile_pool(name="dbf", bufs=4))
        score_pool = ctx.enter_context(tc.tile_pool(name="score", bufs=2, space="PSUM"))
        sm_pool = ctx.enter_context(tc.tile_pool(name="sm", bufs=4))
        prob_pool = ctx.enter_context(tc.tile_pool(name="prob", bufs=2))

        w1_sb = singles.tile([128, KC, U], f32)
        w2_sb = singles.tile([128, KC, U], f32)
        b12_sb = singles.tile([128, 1], f32)
        vcol_sb = singles.tile([128, 1], f32)
        bias1_sb = singles.tile([128, 1], f32)  # FW * b12
        biash_sb = singles.tile([128, 1], f32)  # FW/2 * b12

        # --- input DMAs (descriptor-gen is serial per engine: spread) ---
        dec_sbs, enc_sbs = [], []
        for b in range(BPC):
            dec_sb = proj_in.tile([128, KC, Td], f32, tag="dec")
            dec_sbs.append(dec_sb)
            enc_sb = proj_in.tile([128, KC, Te], f32, tag="enc")
            enc_sbs.append(enc_sb)

        def enc_dma(eng, b, h):
            eng.dma_start(
                out=enc_sbs[b][:, :, h * 512:(h + 1) * 512],
                in_=encT[b, :, :, h * 512:(h + 1) * 512],
            )

        enc_dma(nc.gpsimd, 0, 0)
        enc_dma(nc.scalar, 0, 1)
        nc.sync.dma_start(out=w2_sb, in_=w2[:, :, :])
        nc.sync.dma_start(out=w1_sb, in_=w1[:, :, :])
        nc.sync.dma_start(out=dec_sbs[0], in_=decT[0])
        nc.sync.dma_start(out=dec_sbs[1], in_=decT[1])
        nc.sync.dma_start(out=b12_sb, in_=b12[:, :])
        nc.sync.dma_start(out=vcol_sb, in_=vcol[:, :])
        enc_dma(nc.sync, 1, 0)
        enc_dma(nc.sync, 1, 1)

        nc.vector.tensor_scalar_mul(bias1_sb, b12_sb, FW)
        nc.vector.tensor_scalar_mul(biash_sb, b12_sb, FW / 2)

        # --- projections (PSUM) + base sines via ACT (scale/bias fused) ---
        # e-side: s1e = sin(FW*w2e), she = sin(FW/2*w2e)   [128, Te] f32
        # d-side: s1d = sin(FW*(w1d+b12)), shd likewise    [128, Td] f32
        s1e, she, s1d, shd = [], [], [], []
        for b in range(BPC):
            s1 = base_pool.tile([128, Te], f32, tag="s1e")
            sh = base_pool.tile([128, Te], f32, tag="she")
            for h in range(Te // 512):
                eps = proj_ps.tile([128, 512], f32, tag="encps")
                for c in range(KC):
                    nc.tensor.matmul(
                        eps, w2_sb[:, c, :], enc_sbs[b][:, c, h * 512:(h + 1) * 512],
                        start=(c == 0), stop=(c == KC - 1),
                    )
                sl = slice(h * 512, (h + 1) * 512)
                nc.scalar.activation(s1[:, sl], eps, AF.Sin, scale=FW)
                nc.scalar.activation(sh[:, sl], eps, AF.Sin, scale=FW / 2)
            s1e.append(s1)
            she.append(sh)

            dps = proj_ps.tile([128, Td], f32, tag="decps")
            for c in range(KC):
                nc.tensor.matmul(
                    dps, w1_sb[:, c, :], dec_sbs[b][:, c, :],
                    start=(c == 0), stop=(c == KC - 1),
                )
            sd1 = base_pool.tile([128, Td], f32, tag="s1d")
            sdh = base_pool.tile([128, Td], f32, tag="shd")
            nc.scalar.activation(sd1, dps, AF.Sin, scale=FW, bias=bias1_sb[:, 0:1])
            nc.scalar.activation(sdh, dps, AF.Sin, scale=FW / 2, bias=biash_sb[:, 0:1])
            s1d.append(sd1)
            shd.append(sdh)

        # --- chain state per b: cos1 = 1-2*sh^2, m = 2*cos1 (fp32) ---
        def chain_init(sh_t, n, tag):
            c1 = base_pool.tile([128, n], f32, tag="c1" + tag)
            m = base_pool.tile([128, n], f32, tag="m" + tag)
            nc.vector.tensor_mul(c1, sh_t, sh_t)
            nc.vector.tensor_scalar(c1, c1, -2.0, 1.0, op0=MUL, op1=ADD)
            nc.vector.tensor_scalar_mul(m, c1, 2.0)
            return c1, m

        c1e, me, c1d, md = [], [], [], []
        for b in range(BPC):
            c1, m = chain_init(she[b], Te, "e")
            c1e.append(c1)
            me.append(m)
            c1, m = chain_init(shd[b], Td, "d")
            c1d.append(c1)
            md.append(m)

        sc_ps = [
            score_pool.tile([128, Te], f32, tag="scps", name=f"scps{b}")
            for b in range(BPC)
        ]

        # --- harmonic loop: cast to bf16, fold c_k*V into d-side, matmul,
        # then advance the Chebyshev recurrence (fp32) ---
        es_prev2, ec_prev2 = [None, None], [None, None]
        es_prev1, ec_prev1 = list(s1e), list(c1e)
        ds_prev2, dc_prev2 = [None, None], [None, None]
        ds_prev1, dc_prev1 = list(s1d), list(c1d)

        for k in range(1, FK + 1):
            ck = coef[k - 1]
            for b in range(BPC):
                # bf16 matmul inputs
                Sk = ebf.tile([128, Te], bf16, tag="Sk")
                Qk = ebf.tile([128, Te], bf16, tag="Qk")
                nc.vector.tensor_copy(Sk, es_prev1[b])
                nc.vector.tensor_copy(Qk, ec_prev1[b])
                Pk = dbf.tile([128, Td], bf16, tag="Pk")
                Rk = dbf.tile([128, Td], bf16, tag="Rk")
                nc.vector.tensor_scalar(Pk, ds_prev1[b], vcol_sb[:, 0:1], ck, op0=MUL, op1=MUL)
                nc.vector.tensor_scalar(Rk, dc_prev1[b], vcol_sb[:, 0:1], ck, op0=MUL, op1=MUL)
                for h in range(Te // 512):
                    sl = slice(h * 512, (h + 1) * 512)
                    nc.tensor.matmul(
                        sc_ps[b][:, sl], Pk, Qk[:, sl],
                        start=(k == 1), stop=False, skip_group_check=True,
                    )
                    nc.tensor.matmul(
                        sc_ps[b][:, sl], Rk, Sk[:, sl],
                        start=False, stop=(k == FK), skip_group_check=True,
                    )
                if k < FK:
                    # x_{k+1} = m*x_k - x_{k-1}
                    def step(pool, tag, n, m_t, prev1, prev2):
                        nxt = pool.tile([128, n], f32, tag=tag)
                        nc.vector.tensor_mul(nxt, m_t, prev1)
                        if prev2 is not None:
                            nc.vector.tensor_sub(nxt, nxt, prev2)
                        return nxt

                    if k == 1:
                        # x_2 = m*x_1 - x_0 with x_0 = (0 for sin, 1 for cos)
                        es_nxt = step(echain, f"es{b}", Te, me[b], es_prev1[b], None)
                        ec_nxt = step(echain, f"ec{b}", Te, me[b], ec_prev1[b], None)
                        nc.vector.tensor_scalar_add(ec_nxt, ec_nxt, -1.0)
                        ds_nxt = step(dchain, f"ds{b}", Td, md[b], ds_prev1[b], None)
                        dc_nxt = step(dchain, f"dc{b}", Td, md[b], dc_prev1[b], None)
                        nc.vector.tensor_scalar_add(dc_nxt, dc_nxt, -1.0)
                    else:
                        es_nxt = step(echain, f"es{b}", Te, me[b], es_prev1[b], es_prev2[b])
                        ec_nxt = step(echain, f"ec{b}", Te, me[b], ec_prev1[b], ec_prev2[b])
                        ds_nxt = step(dchain, f"ds{b}", Td, md[b], ds_prev1[b], ds_prev2[b])
                        dc_nxt = step(dchain, f"dc{b}", Td, md[b], dc_prev1[b], dc_prev2[b])
                    es_prev2[b], es_prev1[b] = es_prev1[b], es_nxt
                    ec_prev2[b], ec_prev1[b] = ec_prev1[b], ec_nxt
                    ds_prev2[b], ds_prev1[b] = ds_prev1[b], ds_nxt
                    dc_prev2[b], dc_prev1[b] = dc_prev1[b], dc_nxt

        # --- softmax over s + output (rows are t directly: single DMA) ---
        for b in range(BPC):
            probs = prob_pool.tile([128, Te], f32, tag="probs")
            sums = sm_pool.tile([128, 1], f32, tag="sums")
            nc.scalar.activation(probs, sc_ps[b], AF.Exp, accum_out=sums[:, 0:1])
            rinv = sm_pool.tile([128, 1], f32, tag="rinv")
            nc.vector.reciprocal(rinv, sums)
            nc.vector.tensor_scalar_mul(probs, probs, rinv[:, 0:1])
            (nc.sync if b == 0 else nc.gpsimd).dma_start(out=out[b], in_=probs)

    nc.finalize()
    _NC_CACHE["nc"] = nc
    return nc


def _prep_shards(dec_outputs, enc_outputs, W1, b1, W2, b2, V, bv):
    # [B, T, H] -> [B, 128(p), KC(c), T] with h = c*128 + p
    dec = np.ascontiguousarray(
        np.asarray(dec_outputs, np.float32)
        .transpose(0, 2, 1).reshape(B, KC, 128, Td).transpose(0, 2, 1, 3)
    )
    enc = np.ascontiguousarray(
        np.asarray(enc_outputs, np.float32)
        .transpose(0, 2, 1).reshape(B, KC, 128, Te).transpose(0, 2, 1, 3)
    )
    # [H, U] -> [128(p), KC(c), U]
    w1 = np.ascontiguousarray(np.asarray(W1, np.float32).reshape(KC, 128, U).transpose(1, 0, 2))
    w2 = np.ascontiguousarray(np.asarray(W2, np.float32).reshape(KC, 128, U).transpose(1, 0, 2))
    b12 = (np.asarray(b1, np.float32) + np.asarray(b2, np.float32)).reshape(U, 1)
    vcol = np.ascontiguousarray(np.asarray(V, np.float32).reshape(U, 1))
    # bv dropped: softmax(score + const) == softmax(score).
    in_maps = []
    for i in range(NCORES):
        in_maps.append({
            "enct": np.ascontiguousarray(enc[i * BPC:(i + 1) * BPC]),
            "dect": np.ascontiguousarray(dec[i * BPC:(i + 1) * BPC]),
            "w1": w1,
            "w2": w2,
            "b12": b12,
            "vcol": vcol,
        })
    return in_maps


def kernel(dec_outputs, enc_outputs, W1, b1, W2, b2, V, bv):
    from concourse.bass_utils import run_bass_kernel_spmd

    nc = _build_nc()
    in_maps = _prep_shards(dec_outputs, enc_outputs, W1, b1, W2, b2, V, bv)
    res = run_bass_kernel_spmd(nc, in_maps, list(range(NCORES))).results
    out = np.concatenate([np.asarray(res[i]["out"]) for i in range(NCORES)], axis=0)
    return np.ascontiguousarray(out.astype(np.float32))


# revision 38
# speedup vs baseline: 1.7611x; 1.1881x over previous
"""Trainium2 Bass kernel for PointerAttention (additive/Bahdanau attention scores).

Computes, for full inputs:
    w1d = dec @ W1 + b1                    # [B, Td, U]
    w2e = enc @ W2 + b2                    # [B, Te, U]
    score[b,t,s] = V . tanh(w1d[b,t] + w2e[b,s]) (+ bv, dropped: softmax-shift-invariant)
    out = softmax(score, axis=-1)          # [B, Td, Te]

Shapes: B=16, Td=128, Te=1024, H=256, U=128. Data-parallel over B on 8 cores.

Architecture: Fourier separation. tanh(z) on |z|<=8.8 is approximated to
~1e-3 by a sine series sum_k c_k sin(k*w*z) (w = pi/11, K=14). The angle
addition formula sin(kw(d+e)) = sin(kwd)cos(kwe) + cos(kwd)sin(kwe)
separates decoder and encoder dependencies, so

    score[t,s] = sum_k [ (c_k V sin_k(d))^T cos_k(e) + (c_k V cos_k(d))^T sin_k(e) ]

is a sum of 2K true [128u] contractions on the TensorEngine with full array
utilization — no per-(t,s) elementwise work at all. ScalarE computes only the
base sin(w*x), sin(w*x/2) of the projections (straight off the projection
PSUM, scale/bias fused); higher harmonics come from the Chebyshev recurrence
x_{k+1} = 2cos1*x_k - x_{k-1} on the VectorEngine in fp32 (bf16 recurrence
would amplify rounding ~1.9x/step), cast to bf16 only at the matmul inputs.
Softmax: scores are bounded (|score| <= sum|V| ~ 10) so exp needs no max
subtraction; row sums ride along via the activation accumulator.
"""

import numpy as np

B, Td, Te, H, U = 16, 128, 1024, 256, 128
NCORES = 8
BPC = B // NCORES  # batches per core
KC = H // 128      # contraction chunks

FL = 11.0          # sine series half-period
FK = 14            # number of harmonics
FW = float(np.pi / FL)
# minimax-ish LSQ fit of tanh on [0, 8.8] in basis sin(k*FW*z) (see notes)
_COEF = None


def _fit_coeffs():
    global _COEF
    if _COEF is None:
        z = np.linspace(0, 8.8, 12000)
        A = np.sin(np.outer(z, FW * np.arange(1, FK + 1)))
        wt = np.ones_like(z)
        for _ in range(60):
            c, *_ = np.linalg.lstsq(A * wt[:, None], np.tanh(z) * wt, rcond=None)
            err = A @ c - np.tanh(z)
            wt = wt * (1 + 1.5 * (np.abs(err) / np.abs(err).max()) ** 2)
            wt /= wt.mean()
        _COEF = [float(x) for x in c]
    return _COEF


_NC_CACHE = {}


def _build_nc():
    if "nc" in _NC_CACHE:
        return _NC_CACHE["nc"]

    from contextlib import ExitStack

    import concourse.bacc as bacc
    import concourse.tile as tile
    from concourse import mybir

    f32 = mybir.dt.float32
    bf16 = mybir.dt.bfloat16
    AF = mybir.ActivationFunctionType
    ADD, MUL = mybir.AluOpType.add, mybir.AluOpType.mult
    SUB = mybir.AluOpType.subtract
    coef = _fit_coeffs()

    nc = bacc.Bacc()
    # host pre-lays contraction dim as [partition, chunk]: h = c*128 + p
    encT = nc.dram_tensor("enct", [BPC, 128, KC, Te], f32, kind="ExternalInput")
    decT = nc.dram_tensor("dect", [BPC, 128, KC, Td], f32, kind="ExternalInput")
    w1 = nc.dram_tensor("w1", [128, KC, U], f32, kind="ExternalInput")
    w2 = nc.dram_tensor("w2", [128, KC, U], f32, kind="ExternalInput")
    b12 = nc.dram_tensor("b12", [U, 1], f32, kind="ExternalInput")
    vcol = nc.dram_tensor("vcol", [U, 1], f32, kind="ExternalInput")
    out = nc.dram_tensor("out", [BPC, Td, Te], f32, kind="ExternalOutput")

    with tile.TileContext(nc) as tc, ExitStack() as ctx:
        singles = ctx.enter_context(tc.tile_pool(name="singles", bufs=1))
        proj_in = ctx.enter_context(tc.tile_pool(name="proj_in", bufs=2))
        proj_ps = ctx.enter_context(tc.tile_pool(name="proj_ps", bufs=2, space="PSUM"))
        base_pool = ctx.enter_context(tc.tile_pool(name="base", bufs=2))
        echain = ctx.enter_context(tc.tile_pool(name="echain", bufs=3))
        dchain = ctx.enter_context(tc.tile_pool(name="dchain", bufs=3))
        ebf = ctx.enter_context(tc.tile_pool(name="ebf", bufs=4))
        dbf = ctx.enter_context(tc.tile_pool(name="dbf", bufs=4))
        score_pool = ctx.enter_context(tc.tile_pool(name="score", bufs=2, space="PSUM"))
        sm_pool = ctx.enter_context(tc.tile_pool(name="sm", bufs=4))
        prob_pool = ctx.enter_context(tc.tile_pool(name="prob", bufs=2))

        w1_sb = singles.tile([128, KC, U], f32)
        w2_sb = singles.tile([128, KC, U], f32)
        b12_sb = singles.tile([128, 1], f32)
        vcol_sb = singles.tile([128, 1], f32)
        bias1_sb = singles.tile([128, 1], f32)  # FW * b12
        biash_sb = singles.tile([128, 1], f32)  # FW/2 * b12

        # --- input DMAs (descriptor-gen is serial per engine: spread) ---
        dec_sbs, enc_sbs = [], []
        for b in range(BPC):
            dec_sb = proj_in.tile([128, KC, Td], f32, tag="dec")
            dec_sbs.append(dec_sb)
            enc_sb = proj_in.tile([128, KC, Te], f32, tag="enc")
            enc_sbs.append(enc_sb)

        def enc_dma(eng, b, h):
            eng.dma_start(
                out=enc_sbs[b][:, :, h * 512:(h + 1) * 512],
                in_=encT[b, :, :, h * 512:(h + 1) * 512],
            )

        enc_dma(nc.gpsimd, 0, 0)
        enc_dma(nc.scalar, 0, 1)
        nc.sync.dma_start(out=w2_sb, in_=w2[:, :, :])
        nc.sync.dma_start(out=w1_sb, in_=w1[:, :, :])
        nc.sync.dma_start(out=dec_sbs[0], in_=decT[0])
        nc.sync.dma_start(out=dec_sbs[1], in_=decT[1])
        nc.sync.dma_start(out=b12_sb, in_=b12[:, :])
        nc.sync.dma_start(out=vcol_sb, in_=vcol[:, :])
        enc_dma(nc.sync, 1, 0)
        enc_dma(nc.sync, 1, 1)

        nc.vector.tensor_scalar_mul(bias1_sb, b12_sb, FW)
        nc.vector.tensor_scalar_mul(biash_sb, b12_sb, FW / 2)

        # --- projections (PSUM) + base sines via ACT (scale/bias fused) ---
        # e-side: s1e = sin(FW*w2e), she = sin(FW/2*w2e)   [128, Te] f32
        # d-side: s1d = sin(FW*(w1d+b12)), shd likewise    [128, Td] f32
        s1e, she = [], []
        s1d_all = base_pool.tile([128, BPC, Td], f32, tag="s1d")
        shd_all = base_pool.tile([128, BPC, Td], f32, tag="shd")
        for b in range(BPC):
            s1 = base_pool.tile([128, Te], f32, tag="s1e")
            sh = base_pool.tile([128, Te], f32, tag="she")
            for h in range(Te // 512):
                eps = proj_ps.tile([128, 512], f32, tag="encps")
                for c in range(KC):
                    nc.tensor.matmul(
                        eps, w2_sb[:, c, :], enc_sbs[b][:, c, h * 512:(h + 1) * 512],
                        start=(c == 0), stop=(c == KC - 1),
                    )
                sl = slice(h * 512, (h + 1) * 512)
                nc.scalar.activation(s1[:, sl], eps, AF.Sin, scale=FW)
                nc.scalar.activation(sh[:, sl], eps, AF.Sin, scale=FW / 2)
            s1e.append(s1)
            she.append(sh)

            dps = proj_ps.tile([128, Td], f32, tag="decps")
            for c in range(KC):
                nc.tensor.matmul(
                    dps, w1_sb[:, c, :], dec_sbs[b][:, c, :],
                    start=(c == 0), stop=(c == KC - 1),
                )
            nc.scalar.activation(
                s1d_all[:, b, :], dps, AF.Sin, scale=FW, bias=bias1_sb[:, 0:1]
            )
            nc.scalar.activation(
                shd_all[:, b, :], dps, AF.Sin, scale=FW / 2, bias=biash_sb[:, 0:1]
            )

        # --- chain state per b: cos1 = 1-2*sh^2, m = 2*cos1 (fp32) ---
        def chain_init(sh_t, n, tag):
            c1 = base_pool.tile([128, n], f32, tag="c1" + tag)
            m = base_pool.tile([128, n], f32, tag="m" + tag)
            nc.vector.tensor_mul(c1, sh_t, sh_t)
            nc.vector.tensor_scalar(c1, c1, -2.0, 1.0, op0=MUL, op1=ADD)
            nc.vector.tensor_scalar_mul(m, c1, 2.0)
            return c1, m

        c1e, me = [], []
        for b in range(BPC):
            c1, m = chain_init(she[b], Te, "e")
            c1e.append(c1)
            me.append(m)
        c1d_all = base_pool.tile([128, BPC, Td], f32, tag="c1d")
        md_all = base_pool.tile([128, BPC, Td], f32, tag="md")
        nc.vector.tensor_mul(c1d_all, shd_all, shd_all)
        nc.vector.tensor_scalar(c1d_all, c1d_all, -2.0, 1.0, op0=MUL, op1=ADD)
        nc.vector.tensor_scalar_mul(md_all, c1d_all, 2.0)

        sc_ps = [
            score_pool.tile([128, Te], f32, tag="scps", name=f"scps{b}")
            for b in range(BPC)
        ]

        # --- harmonic loop: cast to bf16, fold c_k*V into d-side, matmul,
        # then advance the Chebyshev recurrence (fp32) ---
        def step(pool, tag, shape, m_t, prev1, prev2):
            # x_{k+1} = m*x_k - x_{k-1}
            nxt = pool.tile(shape, f32, tag=tag, name=tag)
            nc.vector.tensor_mul(nxt, m_t, prev1)
            if prev2 is not None:
                nc.vector.tensor_sub(nxt, nxt, prev2)
            return nxt

        es_prev2, ec_prev2 = [None, None], [None, None]
        es_prev1, ec_prev1 = list(s1e), list(c1e)
        ds_prev2 = dc_prev2 = None
        ds_prev1, dc_prev1 = s1d_all, c1d_all

        for k in range(1, FK + 1):
            ck = coef[k - 1]
            for b in range(BPC):
                # bf16 matmul inputs — cast on the (otherwise idle) ScalarE
                Sk = ebf.tile([128, Te], bf16, tag="Sk")
                Qk = ebf.tile([128, Te], bf16, tag="Qk")
                nc.scalar.copy(Sk, es_prev1[b])
                nc.scalar.copy(Qk, ec_prev1[b])
                Pk = dbf.tile([128, Td], bf16, tag="Pk")
                Rk = dbf.tile([128, Td], bf16, tag="Rk")
                nc.vector.tensor_scalar(
                    Pk, ds_prev1[:, b, :], vcol_sb[:, 0:1], ck, op0=MUL, op1=MUL
                )
                nc.vector.tensor_scalar(
                    Rk, dc_prev1[:, b, :], vcol_sb[:, 0:1], ck, op0=MUL, op1=MUL
                )
                for h in range(Te // 512):
                    sl = slice(h * 512, (h + 1) * 512)
                    nc.tensor.matmul(
                        sc_ps[b][:, sl], Pk, Qk[:, sl],
                        start=(k == 1), stop=False, skip_group_check=True,
                    )
                    nc.tensor.matmul(
                        sc_ps[b][:, sl], Rk, Sk[:, sl],
                        start=False, stop=(k == FK), skip_group_check=True,
                    )
                if k < FK:
                    if k == 1:
                        # x_2 = m*x_1 - x_0 with x_0 = (0 for sin, 1 for cos)
                        es_nxt = step(echain, f"es{b}", [128, Te], me[b], es_prev1[b], None)
                        ec_nxt = step(echain, f"ec{b}", [128, Te], me[b], ec_prev1[b], None)
                        nc.vector.tensor_scalar_add(ec_nxt, ec_nxt, -1.0)
                    else:
                        es_nxt = step(echain, f"es{b}", [128, Te], me[b], es_prev1[b], es_prev2[b])
                        ec_nxt = step(echain, f"ec{b}", [128, Te], me[b], ec_prev1[b], ec_prev2[b])
                    es_prev2[b], es_prev1[b] = es_prev1[b], es_nxt
                    ec_prev2[b], ec_prev1[b] = ec_prev1[b], ec_nxt
            if k < FK:
                dshape = [128, BPC, Td]
                if k == 1:
                    ds_nxt = step(dchain, "ds", dshape, md_all, ds_prev1, None)
                    dc_nxt = step(dchain, "dc", dshape, md_all, dc_prev1, None)
                    nc.vector.tensor_scalar_add(dc_nxt, dc_nxt, -1.0)
                else:
                    ds_nxt = step(dchain, "ds", dshape, md_all, ds_prev1, ds_prev2)
                    dc_nxt = step(dchain, "dc", dshape, md_all, dc_prev1, dc_prev2)
                ds_prev2, ds_prev1 = ds_prev1, ds_nxt
                dc_prev2, dc_prev1 = dc_prev1, dc_nxt

        # --- softmax over s + output (rows are t directly: single DMA) ---
        for b in range(BPC):
            probs = prob_pool.tile([128, Te], f32, tag="probs")
            sums = sm_pool.tile([128, 1], f32, tag="sums")
            nc.scalar.activation(probs, sc_ps[b], AF.Exp, accum_out=sums[:, 0:1])
            rinv = sm_pool.tile([128, 1], f32, tag="rinv")
            nc.vector.reciprocal(rinv, sums)
            nc.vector.tensor_scalar_mul(probs, probs, rinv[:, 0:1])
            (nc.sync if b == 0 else nc.gpsimd).dma_start(out=out[b], in_=probs)

    nc.finalize()
    _NC_CACHE["nc"] = nc
    return nc


def _prep_shards(dec_outputs, enc_outputs, W1, b1, W2, b2, V, bv):
    # [B, T, H] -> [B, 128(p), KC(c), T] with h = c*128 + p
    dec = np.ascontiguousarray(
        np.asarray(dec_outputs, np.float32)
        .transpose(0, 2, 1).reshape(B, KC, 128, Td).transpose(0, 2, 1, 3)
    )
    enc = np.ascontiguousarray(
        np.asarray(enc_outputs, np.float32)
        .transpose(0, 2, 1).reshape(B, KC, 128, Te).transpose(0, 2, 1, 3)
    )
    # [H, U] -> [128(p), KC(c), U]
    w1 = np.ascontiguousarray(np.asarray(W1, np.float32).reshape(KC, 128, U).transpose(1, 0, 2))
    w2 = np.ascontiguousarray(np.asarray(W2, np.float32).reshape(KC, 128, U).transpose(1, 0, 2))
    b12 = (np.asarray(b1, np.float32) + np.asarray(b2, np.float32)).reshape(U, 1)
    vcol = np.ascontiguousarray(np.asarray(V, np.float32).reshape(U, 1))
    # bv dropped: softmax(score + const) == softmax(score).
    in_maps = []
    for i in range(NCORES):
        in_maps.append({
            "enct": np.ascontiguousarray(enc[i * BPC:(i + 1) * BPC]),
            "dect": np.ascontiguousarray(dec[i * BPC:(i + 1) * BPC]),
            "w1": w1,
            "w2": w2,
            "b12": b12,
            "vcol": vcol,
        })
    return in_maps


def kernel(dec_outputs, enc_outputs, W1, b1, W2, b2, V, bv):
    from concourse.bass_utils import run_bass_kernel_spmd

    nc = _build_nc()
    in_maps = _prep_shards(dec_outputs, enc_outputs, W1, b1, W2, b2, V, bv)
    res = run_bass_kernel_spmd(nc, in_maps, list(range(NCORES))).results
    out = np.concatenate([np.asarray(res[i]["out"]) for i in range(NCORES)], axis=0)
    return np.ascontiguousarray(out.astype(np.float32))
